# revision 1
# baseline (speedup 1.0000x reference)
"""DirectVoxGO render kernel for 8 Trainium2 NeuronCores (Bass/Tile).

Sharding: data-parallel over rays. Host shards samples by contiguous ray
blocks (N_RAYS/8 rays per core), pads each core's chunk to CAP samples,
packs density+k0+mask into one bf16 gather table [V, 16], and builds
per-ray boundary index arrays. Each core runs an identical program:
indirect-DMA brick gathers -> trilinear interp -> alpha -> segmented
compositing via tensor_tensor_scan -> bf16 MLP on TensorE -> per-ray
reduction via global cumsum + boundary gather.
"""

import sys

sys.path.insert(0, "/opt/trn_rl_repo")

from contextlib import ExitStack

import numpy as np
import ml_dtypes

import concourse.bass as bass
import concourse.tile as tile
from concourse import bacc, mybir
from concourse.tile_rust import add_dep_helper

F32 = mybir.dt.float32
BF16 = mybir.dt.bfloat16
I32 = mybir.dt.int32
AX = mybir.AxisListType
OP = mybir.AluOpType
ACTF = mybir.ActivationFunctionType

M_TOT = 524288
N_RAYS = 8192
GX, GY, GZ = 160, 160, 128
NF = 12
POS_PE, VIEW_PE = 10, 4
WIDTH = 128
ACT_SHIFT = float(np.log(1.0 / (1.0 - 1e-6) - 1.0))
P = 128
NCH = 16  # packed grid channels: 12 k0 + density + mask + 2 pad
V = GX * GY * GZ

RNF = 12582912.0  # 1.5 * 2^23: round-to-nearest-int constant
NEG_BIG = -3.0e38
TWOPI = float(2.0 * np.pi)


def _mk(a, dims):
    """Build an AP over a's tensor at a's offset with explicit [step,count] dims."""
    return bass.AP(tensor=a.tensor, offset=a.offset, ap=dims)


class Cfg:
    def __init__(self, F=544, CH=68, RJ=8, GRP=8):
        self.F = F          # free columns/partition (samples/core = 128*F)
        self.CH = CH        # interp chunk columns
        self.RJ = RJ        # ray j-blocks (rays/core = 128*RJ)
        self.GRP = GRP      # MLP group columns
        self.CAP = P * F
        self.RPC = P * RJ
        assert F % CH == 0 and F % GRP == 0


def build_program(cfg: Cfg, dbg=False):
    F, CH, RJ, GRP, CAP = cfg.F, cfg.CH, cfg.RJ, cfg.GRP, cfg.CAP
    nc = bacc.Bacc("TRN2", target_bir_lowering=False, debug=False,
                   enable_asserts=False, num_devices=1)

    xyz_in = nc.dram_tensor("xyz_in", (CAP, 3), F32, kind="ExternalInput")
    validm_in = nc.dram_tensor("validm_in", (CAP,), F32, kind="ExternalInput")
    grid_in = nc.dram_tensor("grid_in", (V, 8 * NCH), BF16, kind="ExternalInput")
    vd_in = nc.dram_tensor("vd_in", (cfg.RPC, 3), F32, kind="ExternalInput")
    w0_in = nc.dram_tensor("w0_in", (75, WIDTH), BF16, kind="ExternalInput")
    w1_in = nc.dram_tensor("w1_in", (WIDTH, WIDTH), BF16, kind="ExternalInput")
    w2_in = nc.dram_tensor("w2_in", (WIDTH, WIDTH), BF16, kind="ExternalInput")
    wrh_in = nc.dram_tensor("wrh_in", (WIDTH, 4), BF16, kind="ExternalInput")
    wrv_in = nc.dram_tensor("wrv_in", (27, 3), F32, kind="ExternalInput")
    brp_in = nc.dram_tensor("brp_in", (3, 1), F32, kind="ExternalInput")
    idf_in = nc.dram_tensor("idf_in", (P, P), F32, kind="ExternalInput")
    idb_in = nc.dram_tensor("idb_in", (P, P), BF16, kind="ExternalInput")
    freq_in = nc.dram_tensor("freq_in", (P, 3 * POS_PE), F32, kind="ExternalInput")
    vfreq_in = nc.dram_tensor("vfreq_in", (P, 3 * VIEW_PE), F32,
                              kind="ExternalInput")
    scl_in = nc.dram_tensor("scl_in", (P, 3), F32, kind="ExternalInput")
    tmax_in = nc.dram_tensor("tmax_in", (P, 3), F32, kind="ExternalInput")
    imax_in = nc.dram_tensor("imax_in", (P, 3), F32, kind="ExternalInput")
    coef_in = nc.dram_tensor("coef_in", (P, 3), F32, kind="ExternalInput")
    spos_in = nc.dram_tensor("spos_in", (P, RJ), I32, kind="ExternalInput")
    epos_in = nc.dram_tensor("epos_in", (P, RJ), I32, kind="ExternalInput")
    ppos_in = nc.dram_tensor("ppos_in", (P, RJ), I32, kind="ExternalInput")

    rgb_out = nc.dram_tensor("rgb_out", (cfg.RPC, 3), F32, kind="ExternalOutput")
    if dbg:
        dbg_ebc = nc.dram_tensor("dbg_ebc", (P, F, 4), F32, kind="ExternalOutput")
        dbg_zc = nc.dram_tensor("dbg_zc", (P, F, 4), F32, kind="ExternalOutput")
        dbg_ze = nc.dram_tensor("dbg_ze", (P, RJ, 4), F32, kind="ExternalOutput")
        dbg_zp = nc.dram_tensor("dbg_zp", (P, RJ, 4), F32, kind="ExternalOutput")
        dbg_l = nc.dram_tensor("dbg_l", (P, F), F32, kind="ExternalOutput")
        dbg_w = nc.dram_tensor("dbg_w", (P, F), F32, kind="ExternalOutput")

    escat = nc.dram_tensor("escat_scr", (CAP, 4), F32, kind="Internal")
    zdram = nc.dram_tensor("zdram_scr", (CAP + 8, 4), F32, kind="Internal")

    with tile.TileContext(nc) as tc, ExitStack() as ctx:
        consts = ctx.enter_context(tc.tile_pool(name="consts", bufs=1))
        planes = ctx.enter_context(tc.tile_pool(name="planes", bufs=1))
        chunkp = ctx.enter_context(tc.tile_pool(name="chunkp", bufs=2))
        mlpp = ctx.enter_context(tc.tile_pool(name="mlpp", bufs=2))
        smallp = ctx.enter_context(tc.tile_pool(name="smallp", bufs=1))
        pst = ctx.enter_context(tc.tile_pool(name="pst", bufs=2, space="PSUM"))
        psh = ctx.enter_context(tc.tile_pool(name="psh", bufs=3, space="PSUM"))
        psr = ctx.enter_context(tc.tile_pool(name="psr", bufs=2, space="PSUM"))

        def cdma(shape, dtype, src):
            t = consts.tile(shape, dtype, name=src.name + "_sb")
            nc.sync.dma_start(out=t[:], in_=src.ap())
            return t

        w0 = cdma([75, WIDTH], BF16, w0_in)
        w1 = cdma([WIDTH, WIDTH], BF16, w1_in)
        w2 = cdma([WIDTH, WIDTH], BF16, w2_in)
        wrh = cdma([WIDTH, 4], BF16, wrh_in)
        wrv = cdma([27, 3], F32, wrv_in)
        brp = cdma([3, 1], F32, brp_in)
        idf = cdma([P, P], F32, idf_in)
        idb = cdma([P, P], BF16, idb_in)
        freqt = cdma([P, 3 * POS_PE], F32, freq_in)
        vfreqt = cdma([P, 3 * VIEW_PE], F32, vfreq_in)
        sclt = cdma([P, 3], F32, scl_in)
        tmaxt = cdma([P, 3], F32, tmax_in)
        imaxt = cdma([P, 3], F32, imax_in)
        coeft = cdma([P, 3], F32, coef_in)
        spost = cdma([P, RJ], I32, spos_in)
        epost = cdma([P, RJ], I32, epos_in)
        ppost = cdma([P, RJ], I32, ppos_in)
        neginf = consts.tile([P, 1], F32)
        nc.vector.memset(neginf[:], NEG_BIG)
        halfpi = consts.tile([P, 1], F32)
        nc.vector.memset(halfpi[:], float(np.pi / 2))
        ashift = consts.tile([P, 1], F32)
        nc.vector.memset(ashift[:], ACT_SHIFT)

        xyzs = planes.tile([P, F, 3], F32)
        nc.sync.dma_start(out=xyzs[:],
                          in_=xyz_in.ap().rearrange("(p f) c -> p f c", p=P))
        validm = planes.tile([P, F], F32)
        nc.sync.dma_start(out=validm[:],
                          in_=validm_in.ap().rearrange("(p f) -> p f", p=P))
        featpl = planes.tile([P, F, 12], BF16)
        log1a = planes.tile([P, F], F32)
        cg = planes.tile([P, F], F32)
        dmx = planes.tile([P, F], F32)
        ebc = planes.tile([P, F, 4], F32)
        zc = planes.tile([P, F, 4], F32)

        grid_ap = grid_in.ap()

        # ---------- per-ray view features (E path) ----------
        vdt = smallp.tile([P, RJ, 3], F32)
        nc.sync.dma_start(out=vdt[:],
                          in_=vd_in.ap().rearrange("(j p) c -> p j c", p=P))
        vpe = smallp.tile([P, RJ, 27], F32)
        nc.vector.tensor_copy(out=vpe[:, :, 0:3], in_=vdt[:])
        vxf = smallp.tile([P, RJ, 3, VIEW_PE], F32)
        vb = vdt[:]
        nc.vector.tensor_tensor(
            out=vxf[:],
            in0=_mk(vb, [*vb.ap, [0, VIEW_PE]]),
            in1=_mk(vfreqt[:], [vfreqt[:].ap[0], [0, RJ], [VIEW_PE, 3],
                                [1, VIEW_PE]]),
            op=OP.mult)
        vxn = smallp.tile([P, RJ, 3, VIEW_PE], F32)
        nc.vector.tensor_scalar(vxn[:], vxf[:], RNF, RNF, OP.add, OP.subtract)
        nc.vector.tensor_tensor(out=vxn[:], in0=vxf[:], in1=vxn[:],
                                op=OP.subtract)
        nc.scalar.activation(out=vpe[:, :, 3:15], in_=vxn[:], func=ACTF.Sin,
                             scale=TWOPI)
        nc.vector.tensor_scalar(vxf[:], vxf[:], 0.25, None, OP.add)
        nc.vector.tensor_scalar(vxn[:], vxf[:], RNF, RNF, OP.add, OP.subtract)
        nc.vector.tensor_tensor(out=vxn[:], in0=vxf[:], in1=vxn[:],
                                op=OP.subtract)
        nc.scalar.activation(out=vpe[:, :, 15:27], in_=vxn[:], func=ACTF.Sin,
                             scale=TWOPI)
        vet = smallp.tile([27, RJ * P], F32)
        for j in range(RJ):
            pvt = pst.tile([27, P], F32, tag="pt", name=f"pvt{j}")
            nc.tensor.transpose(out=pvt[:], in_=vpe[:, j, :], identity=idf[:])
            nc.vector.tensor_copy(out=vet[:, j * P:(j + 1) * P], in_=pvt[:])
        ef = smallp.tile([3, RJ * P], F32)
        for c0 in range(0, RJ * P, 512):
            c1 = min(c0 + 512, RJ * P)
            peh = psh.tile([3, 512], F32, tag="ph", name=f"peh{c0}")
            nc.tensor.matmul(peh[:, : c1 - c0], wrv[:], vet[:, c0:c1],
                             start=True, stop=True)
            nc.scalar.activation(out=ef[:, c0:c1], in_=peh[:, : c1 - c0],
                                 func=ACTF.Identity, bias=brp[:])
        def_ = smallp.tile([3, RJ * P], F32)
        nc.vector.tensor_copy(out=def_[:, 0:1], in_=ef[:, 0:1])
        nc.vector.tensor_tensor(out=def_[:, 1:], in0=ef[:, 1:],
                                in1=ef[:, : RJ * P - 1], op=OP.subtract)
        payload = smallp.tile([P, RJ, 4], F32)
        nc.vector.memset(payload[:, :, 3:4], 1.0)
        for j in range(RJ):
            ppt = pst.tile([P, 3], F32, tag="pt", name=f"ppt{j}")
            nc.tensor.transpose(out=ppt[:], in_=def_[:, j * P:(j + 1) * P],
                                identity=idf[:3, :3])
            nc.vector.tensor_copy(out=payload[:, j, 0:3], in_=ppt[:])
        nc.vector.memset(zc[:], 0.0)
        d_zero = nc.sync.dma_start(
            out=escat.ap().rearrange("(p f) c -> p (f c)", p=P),
            in_=_mk(zc[:], [zc[:].ap[0], [1, F * 4]]))
        d_scats = []
        for j in range(RJ):
            d_scat = nc.gpsimd.indirect_dma_start(
                out=escat.ap(),
                out_offset=bass.IndirectOffsetOnAxis(ap=spost[:, j:j + 1],
                                                     axis=0),
                in_=payload[:, j, :],
                in_offset=None,
                compute_op=OP.add)
            add_dep_helper(d_scat.ins, d_zero.ins, reason="zero before scatter")
            d_scats.append(d_scat)
        d_ebc = nc.sync.dma_start(
            out=ebc[:], in_=escat.ap().rearrange("(p f) c -> p f c", p=P))
        for d_scat in d_scats:
            add_dep_helper(d_ebc.ins, d_scat.ins, reason="scatter before readback")
        ert = smallp.tile([P, 3], F32)
        for c in range(3):
            nc.vector.reduce_sum(out=ert[:, c:c + 1], in_=ebc[:, :, c], axis=AX.X)
        pert = pst.tile([3, P], F32, tag="pt")
        nc.tensor.transpose(out=pert[:], in_=ert[:], identity=idf[:])
        erts = smallp.tile([3, P], F32)
        nc.vector.tensor_copy(out=erts[:], in_=pert[:])
        ecs = smallp.tile([3, P], F32)
        nc.vector.tensor_tensor_scan(out=ecs[:], data0=erts[:], data1=erts[:],
                                     initial=0.0, op0=OP.add, op1=OP.bypass)
        eshift = smallp.tile([3, P], F32)
        nc.vector.memset(eshift[:, 0:1], 0.0)
        nc.vector.tensor_copy(out=eshift[:, 1:], in_=ecs[:, : P - 1])
        pcar = pst.tile([P, 3], F32, tag="pt")
        nc.tensor.transpose(out=pcar[:], in_=eshift[:], identity=idf[:3, :3])
        ecar = smallp.tile([P, 3], F32)
        nc.vector.tensor_copy(out=ecar[:], in_=pcar[:])
        for c in range(3):
            nc.vector.tensor_tensor_scan(
                out=ebc[:, :, c], data0=ebc[:, :, c], data1=ebc[:, :, c],
                initial=ecar[:, c:c + 1], op0=OP.add, op1=OP.bypass)

        # ---------- interpolation chunks ----------
        def bc3(t, n):  # [P,3] const -> [P,n,3]
            a = t[:]
            return _mk(a, [a.ap[0], [0, n], [1, 3]])

        for ci in range(F // CH):
            f0 = ci * CH
            xc = xyzs[:, f0:f0 + CH, :]
            tt = chunkp.tile([P, CH, 3], F32, tag="tt")
            nc.vector.tensor_tensor(out=tt[:], in0=xc, in1=bc3(sclt, CH),
                                    op=OP.mult)
            nc.vector.tensor_tensor(out=tt[:], in0=tt[:], in1=bc3(sclt, CH),
                                    op=OP.add)
            nc.vector.tensor_tensor(out=tt[:], in0=tt[:], in1=bc3(tmaxt, CH),
                                    op=OP.min)
            nc.vector.tensor_scalar(tt[:], tt[:], 0.0, None, OP.max)
            rn = chunkp.tile([P, CH, 3], F32, tag="rn")
            nc.vector.tensor_scalar(rn[:], tt[:], RNF, RNF, OP.add, OP.subtract)
            gt = chunkp.tile([P, CH, 3], F32, tag="gt")
            nc.vector.tensor_tensor(out=gt[:], in0=rn[:], in1=tt[:], op=OP.is_gt)
            nc.vector.tensor_tensor(out=rn[:], in0=rn[:], in1=gt[:],
                                    op=OP.subtract)
            nc.vector.tensor_tensor(out=rn[:], in0=rn[:], in1=bc3(imaxt, CH),
                                    op=OP.min)
            fr = chunkp.tile([P, CH, 3], F32, tag="fr")
            nc.vector.tensor_tensor(out=fr[:], in0=tt[:], in1=rn[:],
                                    op=OP.subtract)
            nc.vector.tensor_tensor(out=gt[:], in0=rn[:], in1=bc3(coeft, CH),
                                    op=OP.mult)
            vf = chunkp.tile([P, CH], F32, tag="vf")
            nc.vector.reduce_sum(out=vf[:], in_=gt[:], axis=AX.X)
            vidx = chunkp.tile([P, CH], I32, tag="vidx")
            nc.vector.tensor_copy(out=vidx[:], in_=vf[:])
            prf = chunkp.tile([P, CH, 3, 2], F32, tag="prf")
            nc.vector.tensor_scalar(prf[:, :, :, 0], fr[:], 1.0, -1.0,
                                    OP.subtract, OP.mult)
            nc.vector.tensor_copy(out=prf[:, :, :, 1], in_=fr[:])
            bb = chunkp.tile([P, CH, 3], F32, tag="bb")
            nc.vector.tensor_scalar(bb[:], fr[:], 0.5, None, OP.is_ge)
            prb = chunkp.tile([P, CH, 3, 2], F32, tag="prb")
            nc.vector.tensor_scalar(prb[:, :, :, 0], bb[:], 1.0, -1.0,
                                    OP.subtract, OP.mult)
            nc.vector.tensor_copy(out=prb[:, :, :, 1], in_=bb[:])

            def pairx(pt):  # x-pair broadcast [P,CH,2,2]
                a = pt[:, :, 0, :]
                return _mk(a, [a.ap[0], a.ap[1], a.ap[2], [0, 2]])

            def pairy(pt):  # y-pair broadcast [P,CH,2,2]
                a = pt[:, :, 1, :]
                return _mk(a, [a.ap[0], a.ap[1], [0, 2], a.ap[2]])

            wxy = chunkp.tile([P, CH, 4], BF16, tag="wxy")
            nc.vector.tensor_tensor(out=wxy[:], in0=pairx(prf), in1=pairy(prf),
                                    op=OP.mult)
            bxy = chunkp.tile([P, CH, 4], BF16, tag="bxy")
            nc.vector.tensor_tensor(out=bxy[:], in0=pairx(prb), in1=pairy(prb),
                                    op=OP.mult)
            cf = chunkp.tile([P, CH, NCH], BF16, tag="cf")
            fz = fr[:, :, 2]
            nc.vector.tensor_copy(out=cf[:, :, 0:13],
                                  in_=_mk(fz, [*fz.ap, [0, 13]]))
            nc.vector.tensor_copy(out=cf[:, :, 13:14], in_=bb[:, :, 2:3])
            bricks = chunkp.tile([P, CH, 4, 2 * NCH], BF16, tag="bricks")
            for fi in range(CH):
                nc.gpsimd.indirect_dma_start(
                    out=_mk(bricks[:, fi, 0, :],
                            [bricks[:, fi, 0, :].ap[0], [1, 4 * 2 * NCH]]),
                    out_offset=None,
                    in_=grid_ap,
                    in_offset=bass.IndirectOffsetOnAxis(ap=vidx[:, fi:fi + 1],
                                                        axis=0))
            dt = chunkp.tile([P, CH, 4, 14], BF16, tag="dt")
            nc.vector.tensor_tensor(out=dt[:], in0=bricks[:, :, :, NCH:NCH + 14],
                                    in1=bricks[:, :, :, 0:14], op=OP.subtract)
            cfa = cf[:]
            cfb = _mk(cfa, [cfa.ap[0], [NCH, CH], [0, 4], [1, 14]])
            nc.vector.tensor_tensor(out=dt[:], in0=dt[:], in1=cfb, op=OP.mult)
            nc.vector.tensor_tensor(out=dt[:], in0=dt[:],
                                    in1=bricks[:, :, :, 0:14], op=OP.add)
            mul = chunkp.tile([P, CH, 4, 13], BF16, tag="mul")
            wa = wxy[:]
            wxyb = _mk(wa, [wa.ap[0], [4, CH], [1, 4], [0, 13]])
            nc.vector.tensor_tensor(out=mul[:], in0=dt[:, :, :, 0:13], in1=wxyb,
                                    op=OP.mult)
            nc.vector.tensor_tensor(out=mul[:, :, 0:2, :], in0=mul[:, :, 0:2, :],
                                    in1=mul[:, :, 2:4, :], op=OP.add)
            nc.vector.tensor_tensor(out=featpl[:, f0:f0 + CH, :],
                                    in0=mul[:, :, 0, 0:12], in1=mul[:, :, 1, 0:12],
                                    op=OP.add)
            dcol = chunkp.tile([P, CH], BF16, tag="dcol")
            nc.vector.tensor_tensor(out=dcol[:],
                                    in0=mul[:, :, 0, 12], in1=mul[:, :, 1, 12],
                                    op=OP.add)
            msk = chunkp.tile([P, CH, 4], F32, tag="msk")
            nc.vector.tensor_tensor(out=msk[:], in0=dt[:, :, :, 13], in1=bxy[:],
                                    op=OP.mult)
            nc.vector.tensor_tensor(out=msk[:, :, 0:2], in0=msk[:, :, 0:2],
                                    in1=msk[:, :, 2:4], op=OP.add)
            mv = chunkp.tile([P, CH], F32, tag="mv")
            nc.vector.tensor_tensor(out=mv[:], in0=msk[:, :, 0], in1=msk[:, :, 1],
                                    op=OP.add)
            ex = chunkp.tile([P, CH], F32, tag="ex")
            nc.scalar.activation(out=ex[:], in_=dcol[:],
                                 func=ACTF.Exp, bias=ashift[:])
            nc.vector.tensor_tensor(out=mv[:], in0=mv[:],
                                    in1=validm[:, f0:f0 + CH], op=OP.mult)
            nc.vector.tensor_tensor(out=log1a[:, f0:f0 + CH], in0=ex[:],
                                    in1=mv[:], op=OP.mult)

        # ---------- compositing scans ----------
        def stitch_add(plane_2d, nchan_view=None):
            """exclusive cross-partition carry for an inclusive row scan"""
            rt1 = smallp.tile([P, 1], F32, tag="rt1")
            nc.vector.reduce_sum(out=rt1[:], in_=plane_2d, axis=AX.X)
            prt = pst.tile([1, P], F32, tag="pt")
            nc.tensor.transpose(out=prt[:], in_=rt1[:], identity=idf[:])
            prts = smallp.tile([1, P], F32, tag="prts")
            nc.vector.tensor_copy(out=prts[:], in_=prt[:])
            rts = smallp.tile([1, P], F32, tag="rts")
            nc.vector.tensor_tensor_scan(out=rts[:], data0=prts[:], data1=prts[:],
                                         initial=0.0, op0=OP.add, op1=OP.bypass)
            rtsh = smallp.tile([1, P], F32, tag="rtsh")
            nc.vector.memset(rtsh[:, 0:1], 0.0)
            nc.vector.tensor_copy(out=rtsh[:, 1:], in_=rts[:, : P - 1])
            pc1 = pst.tile([P, 1], F32, tag="pt")
            nc.tensor.transpose(out=pc1[:], in_=rtsh[:], identity=idf[:1, :1])
            car = smallp.tile([P, 1], F32, tag="car")
            nc.vector.tensor_copy(out=car[:], in_=pc1[:])
            return car

        car1 = stitch_add(log1a[:])
        nc.vector.tensor_tensor_scan(out=cg[:], data0=log1a[:], data1=log1a[:],
                                     initial=car1[:], op0=OP.add, op1=OP.bypass)
        nc.vector.tensor_tensor(out=cg[:], in0=cg[:], in1=log1a[:],
                                op=OP.subtract)
        nc.vector.tensor_scalar(dmx[:], cg[:], -1.0, None, OP.mult)
        mrk = planes.tile([P, F], mybir.dt.int8)
        nc.vector.tensor_copy(out=mrk[:], in_=ebc[:, :, 3])
        ninf = neginf[:]
        nc.vector.select(out=dmx[:], mask=mrk[:], on_true=dmx[:],
                         on_false=_mk(ninf, [ninf.ap[0], [0, F]]))
        rt1b = smallp.tile([P, 1], F32, tag="rt1")
        nc.vector.reduce_max(out=rt1b[:], in_=dmx[:], axis=AX.X)
        prtb = pst.tile([1, P], F32, tag="pt")
        nc.tensor.transpose(out=prtb[:], in_=rt1b[:], identity=idf[:])
        prtbs = smallp.tile([1, P], F32, tag="prts")
        nc.vector.tensor_copy(out=prtbs[:], in_=prtb[:])
        rtsb = smallp.tile([1, P], F32, tag="rts")
        nc.vector.tensor_tensor_scan(out=rtsb[:], data0=prtbs[:], data1=prtbs[:],
                                     initial=NEG_BIG, op0=OP.max, op1=OP.bypass)
        rtshb = smallp.tile([1, P], F32, tag="rtsh")
        nc.vector.memset(rtshb[:, 0:1], NEG_BIG)
        nc.vector.tensor_copy(out=rtshb[:, 1:], in_=rtsb[:, : P - 1])
        pc1b = pst.tile([P, 1], F32, tag="pt")
        nc.tensor.transpose(out=pc1b[:], in_=rtshb[:], identity=idf[:1, :1])
        carm = smallp.tile([P, 1], F32, tag="car")
        nc.vector.tensor_copy(out=carm[:], in_=pc1b[:])
        nc.vector.tensor_tensor_scan(out=dmx[:], data0=dmx[:], data1=dmx[:],
                                     initial=carm[:], op0=OP.max, op1=OP.bypass)
        nc.vector.tensor_tensor(out=cg[:], in0=cg[:], in1=dmx[:], op=OP.add)
        nc.scalar.activation(out=cg[:], in_=cg[:], func=ACTF.Exp)
        nc.vector.tensor_tensor(out=cg[:], in0=cg[:], in1=log1a[:],
                                op=OP.mult)

        # ---------- MLP ----------
        nsg = GRP * P
        for g in range(F // GRP):
            f0 = g * GRP
            xt_ = mlpp.tile([P, GRP, 75], BF16, tag="xt")
            nc.vector.tensor_copy(out=xt_[:, :, 0:12],
                                  in_=featpl[:, f0:f0 + GRP, :])
            nc.vector.tensor_copy(out=xt_[:, :, 12:15],
                                  in_=xyzs[:, f0:f0 + GRP, :])
            xf = mlpp.tile([P, GRP, 3, POS_PE], F32, tag="xf")
            xa = xyzs[:, f0:f0 + GRP, :]
            nc.vector.tensor_tensor(
                out=xf[:],
                in0=_mk(xa, [*xa.ap, [0, POS_PE]]),
                in1=_mk(freqt[:], [freqt[:].ap[0], [0, GRP], [POS_PE, 3],
                                   [1, POS_PE]]),
                op=OP.mult)
            xn = mlpp.tile([P, GRP, 3, POS_PE], F32, tag="xn")
            nc.vector.tensor_scalar(xn[:], xf[:], RNF, RNF, OP.add, OP.subtract)
            nc.vector.tensor_tensor(out=xn[:], in0=xf[:], in1=xn[:],
                                    op=OP.subtract)
            nc.scalar.activation(out=xt_[:, :, 15:45], in_=xn[:], func=ACTF.Sin,
                                 scale=TWOPI)
            nc.vector.tensor_scalar(xf[:], xf[:], 0.25, None, OP.add)
            nc.vector.tensor_scalar(xn[:], xf[:], RNF, RNF, OP.add, OP.subtract)
            nc.vector.tensor_tensor(out=xn[:], in0=xf[:], in1=xn[:],
                                    op=OP.subtract)
            nc.scalar.activation(out=xt_[:, :, 45:75], in_=xn[:], func=ACTF.Sin,
                                 scale=TWOPI)
            xT = mlpp.tile([75, nsg], BF16, tag="xT")
            for i in range(GRP):
                pxt = pst.tile([75, P], BF16, tag="pt", name=f"pxt{g}_{i}")
                nc.tensor.transpose(out=pxt[:], in_=xt_[:, i, :], identity=idb[:])
                if i % 2 == 0:
                    nc.vector.tensor_copy(out=xT[:, i * P:(i + 1) * P], in_=pxt[:])
                else:
                    nc.scalar.copy(out=xT[:, i * P:(i + 1) * P], in_=pxt[:])
            a0 = mlpp.tile([P, nsg], BF16, tag="a0")
            a1 = mlpp.tile([P, nsg], BF16, tag="a1")
            a2 = mlpp.tile([P, nsg], BF16, tag="a2")
            for h in range(nsg // 512):
                c0 = h * 512
                ph0 = psh.tile([P, 512], F32, tag="ph", name=f"ph0_{g}_{h}")
                nc.tensor.matmul(ph0[:], w0[:], xT[:, c0:c0 + 512],
                                 start=True, stop=True)
                nc.scalar.activation(out=a0[:, c0:c0 + 512], in_=ph0[:],
                                     func=ACTF.Relu)
                ph1 = psh.tile([P, 512], F32, tag="ph", name=f"ph1_{g}_{h}")
                nc.tensor.matmul(ph1[:], w1[:], a0[:, c0:c0 + 512],
                                 start=True, stop=True)
                nc.vector.tensor_scalar(a1[:, c0:c0 + 512], ph1[:], 0.0, None,
                                        OP.max)
                ph2 = psh.tile([P, 512], F32, tag="ph", name=f"ph2_{g}_{h}")
                nc.tensor.matmul(ph2[:], w2[:], a1[:, c0:c0 + 512],
                                 start=True, stop=True)
                nc.scalar.activation(out=a2[:, c0:c0 + 512], in_=ph2[:],
                                     func=ACTF.Relu)
            prgb = psr.tile([P, GRP, 4], F32, tag="pr", name=f"prgb{g}")
            for i in range(GRP):
                nc.tensor.matmul(prgb[:, i, :], a2[:, i * P:(i + 1) * P], wrh[:],
                                 start=True, stop=True)
            sgarg = mlpp.tile([P, GRP, 3], F32, tag="sgarg")
            nc.vector.tensor_tensor(out=sgarg[:], in0=prgb[:, :, 0:3],
                                    in1=ebc[:, f0:f0 + GRP, 0:3], op=OP.add)
            nc.scalar.activation(out=zc[:, f0:f0 + GRP, 0:3], in_=sgarg[:],
                                 func=ACTF.Sigmoid)

        # ---------- weighted per-ray sums ----------
        ta = cg[:]
        nc.vector.tensor_tensor(out=zc[:, :, 0:3], in0=zc[:, :, 0:3],
                                in1=_mk(ta, [*ta.ap, [0, 3]]), op=OP.mult)
        nc.vector.tensor_copy(out=zc[:, :, 3], in_=log1a[:])
        zrt = smallp.tile([P, 4], F32)
        for c in range(4):
            nc.vector.reduce_sum(out=zrt[:, c:c + 1], in_=zc[:, :, c], axis=AX.X)
        pzrt = pst.tile([4, P], F32, tag="pt")
        nc.tensor.transpose(out=pzrt[:], in_=zrt[:], identity=idf[:])
        pzrts = smallp.tile([4, P], F32)
        nc.vector.tensor_copy(out=pzrts[:], in_=pzrt[:])
        zcs = smallp.tile([4, P], F32)
        nc.vector.tensor_tensor_scan(out=zcs[:], data0=pzrts[:], data1=pzrts[:],
                                     initial=0.0, op0=OP.add, op1=OP.bypass)
        zsh = smallp.tile([4, P], F32)
        nc.vector.memset(zsh[:, 0:1], 0.0)
        nc.vector.tensor_copy(out=zsh[:, 1:], in_=zcs[:, : P - 1])
        pzc = pst.tile([P, 4], F32, tag="pt")
        nc.tensor.transpose(out=pzc[:], in_=zsh[:], identity=idf[:4, :4])
        zcar = smallp.tile([P, 4], F32)
        nc.vector.tensor_copy(out=zcar[:], in_=pzc[:])
        for c in range(4):
            nc.vector.tensor_tensor_scan(
                out=zc[:, :, c], data0=zc[:, :, c], data1=zc[:, :, c],
                initial=zcar[:, c:c + 1], op0=OP.add, op1=OP.bypass)
        d_z = nc.sync.dma_start(
            out=zdram.ap()[0:CAP, :].rearrange("(p f) c -> p (f c)", p=P),
            in_=zc[:])
        zpad = smallp.tile([8, 4], F32)
        nc.vector.memset(zpad[:], 0.0)
        d_zp = nc.sync.dma_start(out=zdram.ap()[CAP:CAP + 8, :], in_=zpad[:])
        ze = smallp.tile([P, RJ, 4], F32)
        zp = smallp.tile([P, RJ, 4], F32)
        for j in range(RJ):
            d_ge = nc.gpsimd.indirect_dma_start(
                out=ze[:, j, :], out_offset=None, in_=zdram.ap(),
                in_offset=bass.IndirectOffsetOnAxis(ap=epost[:, j:j + 1], axis=0))
            d_gp = nc.gpsimd.indirect_dma_start(
                out=zp[:, j, :], out_offset=None, in_=zdram.ap(),
                in_offset=bass.IndirectOffsetOnAxis(ap=ppost[:, j:j + 1], axis=0))
            for d_g in (d_ge, d_gp):
                add_dep_helper(d_g.ins, d_z.ins, reason="cumsum before gather")
                add_dep_helper(d_g.ins, d_zp.ins, reason="zpad before gather")
        ainv = smallp.tile([P, RJ], F32)
        nc.vector.tensor_tensor(out=ainv[:], in0=ze[:, :, 3], in1=zp[:, :, 3],
                                op=OP.subtract)
        nc.scalar.activation(out=ainv[:], in_=ainv[:], func=ACTF.Exp)
        outt = smallp.tile([P, RJ, 3], F32)
        nc.vector.tensor_tensor(out=outt[:], in0=zp[:, :, 0:3],
                                in1=ze[:, :, 0:3], op=OP.subtract)
        av = ainv[:]
        nc.vector.tensor_tensor(out=outt[:], in0=outt[:],
                                in1=_mk(av, [*av.ap, [0, 3]]), op=OP.add)
        nc.sync.dma_start(
            out=rgb_out.ap().rearrange("(j p) c -> p j c", p=P), in_=outt[:])
        if dbg:
            nc.sync.dma_start(out=dbg_ebc.ap(), in_=ebc[:])
            d_dzc = nc.sync.dma_start(out=dbg_zc.ap(), in_=zc[:])
            add_dep_helper(d_dzc.ins, d_z.ins, reason="dump zc after zdram write")
            nc.sync.dma_start(out=dbg_ze.ap(), in_=ze[:])
            nc.sync.dma_start(out=dbg_zp.ap(), in_=zp[:])
            nc.sync.dma_start(out=dbg_l.ap(), in_=log1a[:])
            d_dw = nc.sync.dma_start(out=dbg_w.ap(), in_=cg[:])
            add_dep_helper(d_dw.ins, d_z.ins, reason="cg final value")

    nc.compile()
    return nc


# ================= host side =================

def prepare_in_maps(inputs, cfg: Cfg):
    xyz = np.asarray(inputs["xyz"], np.float32)
    ray_id = np.asarray(inputs["ray_id"]).astype(np.int64)
    viewdirs = np.asarray(inputs["viewdirs"], np.float32)
    mask = np.asarray(inputs["mask"])
    density = np.asarray(inputs["density"], np.float32)
    k0 = np.asarray(inputs["k0"], np.float32)
    n_rays = viewdirs.shape[0]
    ncores = n_rays // cfg.RPC
    assert ncores * cfg.RPC == n_rays

    g16 = np.zeros((GX, GY, GZ, NCH), dtype=ml_dtypes.bfloat16)
    g16[..., 0:NF] = k0
    g16[..., NF] = density
    g16[..., NF + 1] = mask.astype(np.float32)
    xe = np.minimum(np.arange(GX) + 1, GX - 1)
    ye = np.minimum(np.arange(GY) + 1, GY - 1)
    zi = np.minimum(np.arange(GZ) + 1, GZ - 1)
    grid = np.empty((GX, GY, GZ, 4, 2, NCH), dtype=ml_dtypes.bfloat16)
    grid[:, :, :, 0, 0] = g16
    grid[:, :, :, 0, 1] = g16[:, :, zi]
    grid[:, :, :, 1, 0] = g16[:, ye]
    grid[:, :, :, 1, 1] = g16[:, ye][:, :, zi]
    grid[:, :, :, 2, 0] = g16[xe]
    grid[:, :, :, 2, 1] = g16[xe][:, :, zi]
    grid[:, :, :, 3, 0] = g16[xe][:, ye]
    grid[:, :, :, 3, 1] = g16[xe][:, ye][:, :, zi]
    grid = grid.reshape(V, 8 * NCH)

    gb = np.searchsorted(ray_id, np.arange(n_rays + 1))
    w0 = np.asarray(inputs["W0"], np.float32)
    w1 = np.asarray(inputs["W1"], np.float32)
    w2 = np.asarray(inputs["W2"], np.float32)
    wr = np.asarray(inputs["Wr"], np.float32)
    br = np.asarray(inputs["br"], np.float32)
    wrh = np.zeros((WIDTH, 4), np.float32)
    wrh[:, 0:3] = wr[0:WIDTH, :]
    wrv = wr[WIDTH:, :]

    ident = np.eye(P, dtype=np.float32)
    f1 = (2.0 ** np.arange(POS_PE, dtype=np.float64) / (2 * np.pi)
          ).astype(np.float32)
    freqt = np.tile(np.repeat(f1[None, :], 3, axis=0).reshape(1, -1), (P, 1))
    f2 = (2.0 ** np.arange(VIEW_PE, dtype=np.float64) / (2 * np.pi)
          ).astype(np.float32)
    vfreqt = np.tile(np.repeat(f2[None, :], 3, axis=0).reshape(1, -1), (P, 1))
    szf = np.array([GX, GY, GZ], np.float32)

    common = dict(
        grid_in=grid,
        w0_in=w0.astype(ml_dtypes.bfloat16),
        w1_in=w1.astype(ml_dtypes.bfloat16),
        w2_in=w2.astype(ml_dtypes.bfloat16),
        wrh_in=wrh.astype(ml_dtypes.bfloat16),
        wrv_in=np.ascontiguousarray(wrv),
        brp_in=np.ascontiguousarray(br.reshape(3, 1)),
        idf_in=ident,
        idb_in=ident.astype(ml_dtypes.bfloat16),
        freq_in=freqt, vfreq_in=vfreqt,
        scl_in=np.tile(((szf - 1.0) / 2.0)[None, :], (P, 1)),
        tmax_in=np.tile((szf - 1.0)[None, :], (P, 1)),
        imax_in=np.tile((szf - 2.0)[None, :], (P, 1)),
        coef_in=np.tile(np.array([GY * GZ, GZ, 1.0], np.float32)[None, :],
                        (P, 1)),
    )

    def perm(a):  # ray r = j*128+p  ->  [P, RJ] int32
        return np.ascontiguousarray(
            np.asarray(a).reshape(cfg.RJ, P).T).astype(np.int32)

    in_maps = []
    for k in range(ncores):
        b0, b1 = int(gb[k * cfg.RPC]), int(gb[(k + 1) * cfg.RPC])
        nk = b1 - b0
        assert nk <= cfg.CAP, f"core {k}: {nk} samples > CAP {cfg.CAP}"
        xyzp = np.zeros((cfg.CAP, 3), np.float32)
        xyzp[:nk] = xyz[b0:b1]
        validm = np.zeros((cfg.CAP,), np.float32)
        validm[:nk] = -0.5
        rstart = (gb[k * cfg.RPC:(k + 1) * cfg.RPC] - b0).astype(np.int64)
        rend = (gb[k * cfg.RPC + 1:(k + 1) * cfg.RPC + 1] - b0).astype(np.int64)
        epos = rend - 1
        epos[epos < 0] = cfg.CAP
        ppos = rstart - 1
        ppos[ppos < 0] = cfg.CAP
        in_map = dict(common)
        in_map.update(
            xyz_in=xyzp, validm_in=validm,
            vd_in=np.ascontiguousarray(viewdirs[k * cfg.RPC:(k + 1) * cfg.RPC]),
            spos_in=perm(rstart), epos_in=perm(epos), ppos_in=perm(ppos),
        )
        in_maps.append(in_map)
    return in_maps


_PROG_CACHE = {}


def _get_prog(cfg: Cfg):
    key = (cfg.F, cfg.CH, cfg.RJ, cfg.GRP)
    if key not in _PROG_CACHE:
        _PROG_CACHE[key] = build_program(cfg)
    return _PROG_CACHE[key]


def run_on_hw(inputs, cfg=None, trace=False):
    from concourse import bass_utils
    if cfg is None:
        cfg = Cfg()
    nc = _get_prog(cfg)
    in_maps = prepare_in_maps(inputs, cfg)
    res = bass_utils.run_bass_kernel_spmd(
        nc, in_maps, core_ids=list(range(len(in_maps))), trace=trace)
    out = np.concatenate([r["rgb_out"] for r in res.results], axis=0)
    return out, res


def kernel(**inputs) -> np.ndarray:
    out, _ = run_on_hw(inputs)
    return out



# revision 12
# speedup vs baseline: 1.3420x; 1.3420x over previous
"""DirectVoxGO render kernel for 8 Trainium2 NeuronCores (Bass/Tile).

Sharding: data-parallel over rays. Host shards samples by contiguous ray
blocks (N_RAYS/8 rays per core), pads each core's chunk to CAP samples,
packs density+k0+mask into one bf16 gather table [V, 128], and builds
per-ray boundary index arrays. Each core runs an identical program:
one merged indirect-DMA brick gather per chunk -> trilinear interp
(broadcast multiplies on GpSimd) -> interleaved per-group MLP (xbar-DMA
transposes, TensorE matmuls) -> segmented compositing via
tensor_tensor_scan -> per-ray reduction via global cumsum + boundary
gather. Sigmoid/exp batched to avoid ACT table-set thrash.
"""

import sys

sys.path.insert(0, "/opt/trn_rl_repo")

from contextlib import ExitStack

import numpy as np
import ml_dtypes

import concourse.bass as bass
import concourse.tile as tile
from concourse import bacc, mybir
from concourse.tile_rust import add_dep_helper

F32 = mybir.dt.float32
BF16 = mybir.dt.bfloat16
I32 = mybir.dt.int32
AX = mybir.AxisListType
OP = mybir.AluOpType
ACTF = mybir.ActivationFunctionType

M_TOT = 524288
N_RAYS = 8192
GX, GY, GZ = 160, 160, 128
NF = 12
POS_PE, VIEW_PE = 10, 4
WIDTH = 128
ACT_SHIFT = float(np.log(1.0 / (1.0 - 1e-6) - 1.0))
P = 128
NCH = 16  # packed grid channels: 12 k0 + density + mask + 2 pad
V = GX * GY * GZ

RNF = 12582912.0  # 1.5 * 2^23: round-to-nearest-int constant
NEG_BIG = -3.0e38
TWOPI = float(2.0 * np.pi)


def _mk(a, dims):
    """Build an AP over a's tensor at a's offset with explicit [step,count] dims."""
    return bass.AP(tensor=a.tensor, offset=a.offset, ap=dims)


class Cfg:
    def __init__(self, F=528, CH=48, RJ=8, GRP=16):
        self.F = F          # free columns/partition (samples/core = 128*F)
        self.CH = CH        # interp chunk columns
        self.RJ = RJ        # ray j-blocks (rays/core = 128*RJ)
        self.GRP = GRP      # MLP group columns
        self.CAP = P * F
        self.RPC = P * RJ
        assert F % CH == 0 and CH % GRP == 0


def build_program(cfg: Cfg):
    F, CH, RJ, GRP, CAP = cfg.F, cfg.CH, cfg.RJ, cfg.GRP, cfg.CAP
    GPC = CH // GRP  # MLP groups per chunk
    NCHUNK = F // CH
    nc = bacc.Bacc("TRN2", target_bir_lowering=False, debug=False,
                   enable_asserts=False, num_devices=1)

    xyz_in = nc.dram_tensor("xyz_in", (CAP, 3), F32, kind="ExternalInput")
    validm_in = nc.dram_tensor("validm_in", (CAP,), F32, kind="ExternalInput")
    grid_in = nc.dram_tensor("grid_in", (V, 8 * NCH), BF16, kind="ExternalInput")
    vd_in = nc.dram_tensor("vd_in", (cfg.RPC, 3), F32, kind="ExternalInput")
    w0_in = nc.dram_tensor("w0_in", (75, WIDTH), BF16, kind="ExternalInput")
    w1_in = nc.dram_tensor("w1_in", (WIDTH, WIDTH), BF16, kind="ExternalInput")
    w2_in = nc.dram_tensor("w2_in", (WIDTH, WIDTH), BF16, kind="ExternalInput")
    wrh_in = nc.dram_tensor("wrh_in", (WIDTH, 4), BF16, kind="ExternalInput")
    wrv_in = nc.dram_tensor("wrv_in", (27, 3), F32, kind="ExternalInput")
    brp_in = nc.dram_tensor("brp_in", (3, 1), F32, kind="ExternalInput")
    idf_in = nc.dram_tensor("idf_in", (P, P), F32, kind="ExternalInput")
    idb_in = nc.dram_tensor("idb_in", (P, P), BF16, kind="ExternalInput")
    freq_in = nc.dram_tensor("freq_in", (P, 3 * POS_PE), F32, kind="ExternalInput")
    vfreq_in = nc.dram_tensor("vfreq_in", (P, 3 * VIEW_PE), F32,
                              kind="ExternalInput")
    scl_in = nc.dram_tensor("scl_in", (P, 3), F32, kind="ExternalInput")
    tmax_in = nc.dram_tensor("tmax_in", (P, 3), F32, kind="ExternalInput")
    imax_in = nc.dram_tensor("imax_in", (P, 3), F32, kind="ExternalInput")
    coef_in = nc.dram_tensor("coef_in", (P, 3), F32, kind="ExternalInput")
    spos_in = nc.dram_tensor("spos_in", (P, RJ), I32, kind="ExternalInput")
    epos_in = nc.dram_tensor("epos_in", (P, RJ), I32, kind="ExternalInput")
    ppos_in = nc.dram_tensor("ppos_in", (P, RJ), I32, kind="ExternalInput")

    rgb_out = nc.dram_tensor("rgb_out", (cfg.RPC, 3), F32, kind="ExternalOutput")

    escat = nc.dram_tensor("escat_scr", (CAP, 4), F32, kind="Internal")
    zdram = nc.dram_tensor("zdram_scr", (CAP + 8, 4), F32, kind="Internal")

    with tile.TileContext(nc) as tc, ExitStack() as ctx:
        consts = ctx.enter_context(tc.tile_pool(name="consts", bufs=1))
        planes = ctx.enter_context(tc.tile_pool(name="planes", bufs=1))
        chunkp = ctx.enter_context(tc.tile_pool(name="chunkp", bufs=2))
        mlpp = ctx.enter_context(tc.tile_pool(name="mlpp", bufs=2))
        smallp = ctx.enter_context(tc.tile_pool(name="smallp", bufs=1))
        pst = ctx.enter_context(tc.tile_pool(name="pst", bufs=2, space="PSUM"))
        psh = ctx.enter_context(tc.tile_pool(name="psh", bufs=2, space="PSUM"))
        psr = ctx.enter_context(tc.tile_pool(name="psr", bufs=2, space="PSUM"))

        def cdma(shape, dtype, src):
            t = consts.tile(shape, dtype, name=src.name + "_sb")
            nc.sync.dma_start(out=t[:], in_=src.ap())
            return t

        w0 = cdma([75, WIDTH], BF16, w0_in)
        w1 = cdma([WIDTH, WIDTH], BF16, w1_in)
        w2 = cdma([WIDTH, WIDTH], BF16, w2_in)
        wrh = cdma([WIDTH, 4], BF16, wrh_in)
        wrv = cdma([27, 3], F32, wrv_in)
        brp = cdma([3, 1], F32, brp_in)
        idf = cdma([P, P], F32, idf_in)
        idb = cdma([P, P], BF16, idb_in)
        freqt = cdma([P, 3 * POS_PE], F32, freq_in)
        vfreqt = cdma([P, 3 * VIEW_PE], F32, vfreq_in)
        sclt = cdma([P, 3], F32, scl_in)
        tmaxt = cdma([P, 3], F32, tmax_in)
        imaxt = cdma([P, 3], F32, imax_in)
        coeft = cdma([P, 3], F32, coef_in)
        spost = cdma([P, RJ], I32, spos_in)
        epost = cdma([P, RJ], I32, epos_in)
        ppost = cdma([P, RJ], I32, ppos_in)
        neginf = consts.tile([P, 1], F32)
        nc.vector.memset(neginf[:], NEG_BIG)
        ashift = consts.tile([P, 1], F32)
        nc.vector.memset(ashift[:], ACT_SHIFT)

        xyzs = planes.tile([P, F, 3], F32)
        nc.sync.dma_start(out=xyzs[:],
                          in_=xyz_in.ap().rearrange("(p f) c -> p f c", p=P))
        validm = planes.tile([P, F], F32)
        nc.sync.dma_start(out=validm[:],
                          in_=validm_in.ap().rearrange("(p f) -> p f", p=P))
        log1a = planes.tile([P, F], F32)
        cg = planes.tile([P, F], F32)
        dmx = planes.tile([P, F], F32)
        ebc = planes.tile([P, F, 4], F32)
        zc = planes.tile([P, F, 4], F32)
        dcolp = planes.tile([P, F], BF16)
        mvp = planes.tile([P, F], F32)

        grid_ap = grid_in.ap()

        # ---------- per-ray view features (E path) ----------
        vdt = smallp.tile([P, RJ, 3], F32)
        nc.sync.dma_start(out=vdt[:],
                          in_=vd_in.ap().rearrange("(j p) c -> p j c", p=P))
        vpe = smallp.tile([P, RJ, 27], F32)
        nc.vector.tensor_copy(out=vpe[:, :, 0:3], in_=vdt[:])
        vxf = smallp.tile([P, RJ, 3, VIEW_PE], F32)
        vb = vdt[:]
        nc.vector.tensor_tensor(
            out=vxf[:],
            in0=_mk(vb, [*vb.ap, [0, VIEW_PE]]),
            in1=_mk(vfreqt[:], [vfreqt[:].ap[0], [0, RJ], [VIEW_PE, 3],
                                [1, VIEW_PE]]),
            op=OP.mult)
        vxn = smallp.tile([P, RJ, 3, VIEW_PE], F32)
        nc.vector.tensor_scalar(vxn[:], vxf[:], RNF, RNF, OP.add, OP.subtract)
        nc.vector.tensor_tensor(out=vxn[:], in0=vxf[:], in1=vxn[:],
                                op=OP.subtract)
        nc.scalar.activation(out=vpe[:, :, 3:15], in_=vxn[:], func=ACTF.Sin,
                             scale=TWOPI)
        nc.vector.tensor_scalar(vxf[:], vxf[:], 0.25, None, OP.add)
        nc.vector.tensor_scalar(vxn[:], vxf[:], RNF, RNF, OP.add, OP.subtract)
        nc.vector.tensor_tensor(out=vxn[:], in0=vxf[:], in1=vxn[:],
                                op=OP.subtract)
        nc.scalar.activation(out=vpe[:, :, 15:27], in_=vxn[:], func=ACTF.Sin,
                             scale=TWOPI)
        vet = smallp.tile([27, RJ * P], F32)
        for j in range(RJ):
            pvt = pst.tile([27, P], F32, tag="pt", name=f"pvt{j}")
            nc.tensor.transpose(out=pvt[:], in_=vpe[:, j, :], identity=idf[:])
            nc.vector.tensor_copy(out=vet[:, j * P:(j + 1) * P], in_=pvt[:])
        ef = smallp.tile([3, RJ * P], F32)
        for c0 in range(0, RJ * P, 512):
            c1 = min(c0 + 512, RJ * P)
            peh = psh.tile([3, 512], F32, tag="ph", name=f"peh{c0}")
            nc.tensor.matmul(peh[:, : c1 - c0], wrv[:], vet[:, c0:c1],
                             start=True, stop=True)
            nc.scalar.activation(out=ef[:, c0:c1], in_=peh[:, : c1 - c0],
                                 func=ACTF.Identity, bias=brp[:])
        def_ = smallp.tile([3, RJ * P], F32)
        nc.vector.tensor_copy(out=def_[:, 0:1], in_=ef[:, 0:1])
        nc.vector.tensor_tensor(out=def_[:, 1:], in0=ef[:, 1:],
                                in1=ef[:, : RJ * P - 1], op=OP.subtract)
        payload = smallp.tile([P, RJ, 4], F32)
        nc.vector.memset(payload[:, :, 3:4], 1.0)
        for j in range(RJ):
            ppt = pst.tile([P, 3], F32, tag="pt", name=f"ppt{j}")
            nc.tensor.transpose(out=ppt[:], in_=def_[:, j * P:(j + 1) * P],
                                identity=idf[:3, :3])
            nc.vector.tensor_copy(out=payload[:, j, 0:3], in_=ppt[:])
        nc.vector.memset(zc[:], 0.0)
        d_zero = nc.sync.dma_start(
            out=escat.ap().rearrange("(p f) c -> p (f c)", p=P),
            in_=_mk(zc[:], [zc[:].ap[0], [1, F * 4]]))
        d_scats = []
        for j in range(RJ):
            d_scat = nc.gpsimd.indirect_dma_start(
                out=escat.ap(),
                out_offset=bass.IndirectOffsetOnAxis(ap=spost[:, j:j + 1],
                                                     axis=0),
                in_=payload[:, j, :],
                in_offset=None,
                compute_op=OP.add)
            add_dep_helper(d_scat.ins, d_zero.ins, reason="zero before scatter")
            d_scats.append(d_scat)
        d_ebc = nc.sync.dma_start(
            out=ebc[:], in_=escat.ap().rearrange("(p f) c -> p f c", p=P))
        for d_scat in d_scats:
            add_dep_helper(d_ebc.ins, d_scat.ins, reason="scatter before readback")
        ert = smallp.tile([P, 3], F32)
        for c in range(3):
            nc.vector.reduce_sum(out=ert[:, c:c + 1], in_=ebc[:, :, c], axis=AX.X)
        pert = pst.tile([3, P], F32, tag="pt")
        nc.tensor.transpose(out=pert[:], in_=ert[:], identity=idf[:])
        erts = smallp.tile([3, P], F32)
        nc.vector.tensor_copy(out=erts[:], in_=pert[:])
        ecs = smallp.tile([3, P], F32)
        nc.vector.tensor_tensor_scan(out=ecs[:], data0=erts[:], data1=erts[:],
                                     initial=0.0, op0=OP.add, op1=OP.bypass)
        eshift = smallp.tile([3, P], F32)
        nc.vector.memset(eshift[:, 0:1], 0.0)
        nc.vector.tensor_copy(out=eshift[:, 1:], in_=ecs[:, : P - 1])
        pcar = pst.tile([P, 3], F32, tag="pt")
        nc.tensor.transpose(out=pcar[:], in_=eshift[:], identity=idf[:3, :3])
        ecar = smallp.tile([P, 3], F32)
        nc.vector.tensor_copy(out=ecar[:], in_=pcar[:])
        for c in range(3):
            nc.vector.tensor_tensor_scan(
                out=ebc[:, :, c], data0=ebc[:, :, c], data1=ebc[:, :, c],
                initial=ecar[:, c:c + 1], op0=OP.add, op1=OP.bypass)

        # ---------- broadcast helpers ----------
        def bc3(t, n):  # [P,3] const -> [P,n,3]
            a = t[:]
            return _mk(a, [a.ap[0], [0, n], [1, 3]])

        def pairx(pt):  # x-pair broadcast [P,CH,2,2]
            a = pt[:, :, 0, :]
            return _mk(a, [a.ap[0], a.ap[1], a.ap[2], [0, 2]])

        def pairy(pt):  # y-pair broadcast [P,CH,2,2]
            a = pt[:, :, 1, :]
            return _mk(a, [a.ap[0], a.ap[1], [0, 2], a.ap[2]])

        def interp_chunk(ci):
            f0 = ci * CH
            xc = xyzs[:, f0:f0 + CH, :]
            tt = chunkp.tile([P, CH, 3], F32, tag="tt")
            nc.vector.tensor_tensor(out=tt[:], in0=xc, in1=bc3(sclt, CH),
                                    op=OP.mult)
            nc.vector.tensor_tensor(out=tt[:], in0=tt[:], in1=bc3(sclt, CH),
                                    op=OP.add)
            nc.vector.tensor_tensor(out=tt[:], in0=tt[:], in1=bc3(tmaxt, CH),
                                    op=OP.min)
            nc.vector.tensor_scalar(tt[:], tt[:], 0.0, None, OP.max)
            rn = chunkp.tile([P, CH, 3], F32, tag="rn")
            nc.vector.tensor_scalar(rn[:], tt[:], RNF, RNF, OP.add, OP.subtract)
            gt = chunkp.tile([P, CH, 3], F32, tag="gt")
            nc.vector.tensor_tensor(out=gt[:], in0=rn[:], in1=tt[:], op=OP.is_gt)
            nc.vector.tensor_tensor(out=rn[:], in0=rn[:], in1=gt[:],
                                    op=OP.subtract)
            nc.vector.tensor_tensor(out=rn[:], in0=rn[:], in1=bc3(imaxt, CH),
                                    op=OP.min)
            fr = chunkp.tile([P, CH, 3], F32, tag="fr")
            nc.vector.tensor_tensor(out=fr[:], in0=tt[:], in1=rn[:],
                                    op=OP.subtract)
            nc.vector.tensor_tensor(out=gt[:], in0=rn[:], in1=bc3(coeft, CH),
                                    op=OP.mult)
            vf = chunkp.tile([P, CH], F32, tag="vf")
            nc.vector.reduce_sum(out=vf[:], in_=gt[:], axis=AX.X)
            vidx = chunkp.tile([P, CH], I32, tag="vidx")
            nc.vector.tensor_copy(out=vidx[:], in_=vf[:])
            prf = chunkp.tile([P, CH, 3, 2], F32, tag="prf")
            nc.vector.tensor_scalar(prf[:, :, :, 0], fr[:], 1.0, -1.0,
                                    OP.subtract, OP.mult)
            nc.vector.tensor_copy(out=prf[:, :, :, 1], in_=fr[:])
            bb = chunkp.tile([P, CH, 3], F32, tag="bb")
            nc.vector.tensor_scalar(bb[:], fr[:], 0.5, None, OP.is_ge)
            prb = chunkp.tile([P, CH, 3, 2], F32, tag="prb")
            nc.vector.tensor_scalar(prb[:, :, :, 0], bb[:], 1.0, -1.0,
                                    OP.subtract, OP.mult)
            nc.vector.tensor_copy(out=prb[:, :, :, 1], in_=bb[:])
            wxy = chunkp.tile([P, CH, 4], BF16, tag="wxy")
            nc.vector.tensor_tensor(out=wxy[:], in0=pairx(prf), in1=pairy(prf),
                                    op=OP.mult)
            bxy = chunkp.tile([P, CH, 4], F32, tag="bxy")
            nc.vector.tensor_tensor(out=bxy[:], in0=pairx(prb), in1=pairy(prb),
                                    op=OP.mult)
            cf = chunkp.tile([P, CH, NCH], BF16, tag="cf")
            fz = fr[:, :, 2]
            nc.vector.tensor_copy(out=cf[:, :, 0:13],
                                  in_=_mk(fz, [*fz.ap, [0, 13]]))
            nc.vector.tensor_copy(out=cf[:, :, 13:14], in_=bb[:, :, 2:3])
            bricks = chunkp.tile([P, CH, 4, 2 * NCH], BF16, tag="bricks")
            for fi in range(CH):
                nc.gpsimd.indirect_dma_start(
                    out=_mk(bricks[:, fi, 0, :],
                            [bricks[:, fi, 0, :].ap[0], [1, 4 * 2 * NCH]]),
                    out_offset=None,
                    in_=grid_ap,
                    in_offset=bass.IndirectOffsetOnAxis(ap=vidx[:, fi:fi + 1],
                                                        axis=0))
            dt = chunkp.tile([P, CH, 4, 14], BF16, tag="dt")
            nc.vector.tensor_tensor(out=dt[:], in0=bricks[:, :, :, NCH:NCH + 14],
                                    in1=bricks[:, :, :, 0:14], op=OP.subtract)
            cfa = cf[:]
            cfb = _mk(cfa, [cfa.ap[0], [NCH, CH], [0, 4], [1, 14]])
            nc.vector.tensor_tensor(out=dt[:], in0=dt[:], in1=cfb, op=OP.mult)
            nc.vector.tensor_tensor(out=dt[:], in0=dt[:],
                                    in1=bricks[:, :, :, 0:14], op=OP.add)
            msk = chunkp.tile([P, CH, 4], F32, tag="msk")
            nc.vector.tensor_tensor(out=msk[:], in0=dt[:, :, :, 13], in1=bxy[:],
                                    op=OP.mult)
            nc.vector.tensor_tensor(out=msk[:, :, 0:2], in0=msk[:, :, 0:2],
                                    in1=msk[:, :, 2:4], op=OP.add)
            mv = chunkp.tile([P, CH], F32, tag="mv")
            nc.vector.tensor_tensor(out=mv[:], in0=msk[:, :, 0], in1=msk[:, :, 1],
                                    op=OP.add)
            nc.vector.tensor_tensor(out=mvp[:, f0:f0 + CH], in0=mv[:],
                                    in1=validm[:, f0:f0 + CH], op=OP.mult)
            wa = wxy[:]
            wxyb = _mk(wa, [wa.ap[0], [4, CH], [1, 4], [0, 13]])
            nc.vector.tensor_tensor(out=dt[:, :, :, 0:13], in0=dt[:, :, :, 0:13],
                                    in1=wxyb, op=OP.mult)
            nc.vector.tensor_tensor(out=dt[:, :, 0:2, 0:13],
                                    in0=dt[:, :, 0:2, 0:13],
                                    in1=dt[:, :, 2:4, 0:13], op=OP.add)
            featpl_c = chunkp.tile([P, CH, 12], BF16, tag="featpl")
            nc.vector.tensor_tensor(out=featpl_c[:],
                                    in0=dt[:, :, 0, 0:12], in1=dt[:, :, 1, 0:12],
                                    op=OP.add)
            nc.vector.tensor_tensor(out=dcolp[:, f0:f0 + CH],
                                    in0=dt[:, :, 0, 12], in1=dt[:, :, 1, 12],
                                    op=OP.add)
            return featpl_c

        # ---------- MLP group ----------
        def mlp_group(g, featpl_c, jloc):
            f0 = g * GRP
            nsg = GRP * P
            xt_ = mlpp.tile([P, GRP, 75], BF16, tag="xt")
            nc.vector.tensor_copy(out=xt_[:, :, 0:12],
                                  in_=featpl_c[:, jloc * GRP:(jloc + 1) * GRP, :])
            nc.vector.tensor_copy(out=xt_[:, :, 12:15],
                                  in_=xyzs[:, f0:f0 + GRP, :])
            args2 = mlpp.tile([P, GRP, 2, 3, POS_PE], F32, tag="args2")
            xa = xyzs[:, f0:f0 + GRP, :]
            nc.vector.tensor_tensor(
                out=args2[:, :, 0],
                in0=_mk(xa, [*xa.ap, [0, POS_PE]]),
                in1=_mk(freqt[:], [freqt[:].ap[0], [0, GRP], [POS_PE, 3],
                                   [1, POS_PE]]),
                op=OP.mult)
            nc.vector.tensor_scalar(args2[:, :, 1], args2[:, :, 0], 0.25, None,
                                    OP.add)
            rn2 = mlpp.tile([P, GRP, 2, 3, POS_PE], F32, tag="rn2")
            nc.vector.tensor_scalar(rn2[:], args2[:], RNF, RNF, OP.add,
                                    OP.subtract)
            nc.vector.tensor_tensor(out=args2[:], in0=args2[:], in1=rn2[:],
                                    op=OP.subtract)
            nc.scalar.activation(out=xt_[:, :, 15:75], in_=args2[:],
                                 func=ACTF.Sin, scale=TWOPI)
            xT = mlpp.tile([75, nsg], BF16, tag="xT")
            for r in range(GRP // 4):
                pxt = pst.tile([75, 4 * P], BF16, tag="ptx", name=f"pxt{g}_{r}")
                for k in range(4):
                    i = r * 4 + k
                    nc.tensor.transpose(out=pxt[:, k * P:(k + 1) * P],
                                        in_=xt_[:, i, :], identity=idb[:])
                if r % 2 == 0:
                    nc.vector.tensor_copy(out=xT[:, r * 512:(r + 1) * 512],
                                          in_=pxt[:])
                else:
                    nc.scalar.copy(out=xT[:, r * 512:(r + 1) * 512], in_=pxt[:])
            a0 = mlpp.tile([P, nsg], BF16, tag="a0")
            a1 = mlpp.tile([P, nsg], BF16, tag="a1")
            a2 = mlpp.tile([P, nsg], BF16, tag="a2")
            for h in range(nsg // 512):
                c0 = h * 512
                ph0 = psh.tile([P, 512], F32, tag="ph", name=f"ph0_{g}_{h}")
                nc.tensor.matmul(ph0[:], w0[:], xT[:, c0:c0 + 512],
                                 start=True, stop=True)
                nc.scalar.activation(out=a0[:, c0:c0 + 512], in_=ph0[:],
                                     func=ACTF.Relu)
                ph1 = psh.tile([P, 512], F32, tag="ph", name=f"ph1_{g}_{h}")
                nc.tensor.matmul(ph1[:], w1[:], a0[:, c0:c0 + 512],
                                 start=True, stop=True)
                if h % 2 == 0:
                    nc.vector.tensor_scalar(a1[:, c0:c0 + 512], ph1[:], 0.0,
                                            None, OP.max)
                else:
                    nc.scalar.activation(out=a1[:, c0:c0 + 512], in_=ph1[:],
                                         func=ACTF.Relu)
                ph2 = psh.tile([P, 512], F32, tag="ph", name=f"ph2_{g}_{h}")
                nc.tensor.matmul(ph2[:], w2[:], a1[:, c0:c0 + 512],
                                 start=True, stop=True)
                nc.scalar.activation(out=a2[:, c0:c0 + 512], in_=ph2[:],
                                     func=ACTF.Relu)
            prgb = psr.tile([P, GRP, 4], F32, tag="pr", name=f"prgb{g}")
            for i in range(GRP):
                nc.tensor.matmul(prgb[:, i, :], a2[:, i * P:(i + 1) * P], wrh[:],
                                 start=True, stop=True)
            nc.vector.tensor_tensor(out=zc[:, f0:f0 + GRP, 0:3],
                                    in0=prgb[:, :, 0:3],
                                    in1=ebc[:, f0:f0 + GRP, 0:3], op=OP.add)

        # ---------- interleaved interp + MLP ----------
        for ci in range(NCHUNK):
            featpl_c = interp_chunk(ci)
            for j in range(GPC):
                mlp_group(ci * GPC + j, featpl_c, j)

        # ---------- rgb sigmoid (batched, one table load) ----------
        nc.scalar.activation(out=zc[:, :, 0:3], in_=zc[:, :, 0:3],
                             func=ACTF.Sigmoid)

        # ---------- alpha from density (batched exp) ----------
        ex = planes.tile([P, F], F32)
        nc.scalar.activation(out=ex[:], in_=dcolp[:], func=ACTF.Exp,
                             bias=ashift[:])
        nc.vector.tensor_tensor(out=log1a[:], in0=ex[:], in1=mvp[:],
                                op=OP.mult)

        # ---------- compositing scans ----------
        def stitch_add(plane_2d):
            """exclusive cross-partition carry for an inclusive row scan"""
            rt1 = smallp.tile([P, 1], F32, tag="rt1")
            nc.vector.reduce_sum(out=rt1[:], in_=plane_2d, axis=AX.X)
            prt = pst.tile([1, P], F32, tag="pt")
            nc.tensor.transpose(out=prt[:], in_=rt1[:], identity=idf[:])
            prts = smallp.tile([1, P], F32, tag="prts")
            nc.vector.tensor_copy(out=prts[:], in_=prt[:])
            rts = smallp.tile([1, P], F32, tag="rts")
            nc.vector.tensor_tensor_scan(out=rts[:], data0=prts[:], data1=prts[:],
                                         initial=0.0, op0=OP.add, op1=OP.bypass)
            rtsh = smallp.tile([1, P], F32, tag="rtsh")
            nc.vector.memset(rtsh[:, 0:1], 0.0)
            nc.vector.tensor_copy(out=rtsh[:, 1:], in_=rts[:, : P - 1])
            pc1 = pst.tile([P, 1], F32, tag="pt")
            nc.tensor.transpose(out=pc1[:], in_=rtsh[:], identity=idf[:1, :1])
            car = smallp.tile([P, 1], F32, tag="car")
            nc.vector.tensor_copy(out=car[:], in_=pc1[:])
            return car

        car1 = stitch_add(log1a[:])
        nc.vector.tensor_tensor_scan(out=cg[:], data0=log1a[:], data1=log1a[:],
                                     initial=car1[:], op0=OP.add, op1=OP.bypass)
        nc.vector.tensor_tensor(out=cg[:], in0=cg[:], in1=log1a[:],
                                op=OP.subtract)
        nc.vector.tensor_scalar(dmx[:], cg[:], -1.0, None, OP.mult)
        mrk = planes.tile([P, F], mybir.dt.int8)
        nc.vector.tensor_copy(out=mrk[:], in_=ebc[:, :, 3])
        ninf = neginf[:]
        nc.vector.select(out=dmx[:], mask=mrk[:], on_true=dmx[:],
                         on_false=_mk(ninf, [ninf.ap[0], [0, F]]))
        rt1b = smallp.tile([P, 1], F32, tag="rt1")
        nc.vector.reduce_max(out=rt1b[:], in_=dmx[:], axis=AX.X)
        prtb = pst.tile([1, P], F32, tag="pt")
        nc.tensor.transpose(out=prtb[:], in_=rt1b[:], identity=idf[:])
        prtbs = smallp.tile([1, P], F32, tag="prts")
        nc.vector.tensor_copy(out=prtbs[:], in_=prtb[:])
        rtsb = smallp.tile([1, P], F32, tag="rts")
        nc.vector.tensor_tensor_scan(out=rtsb[:], data0=prtbs[:], data1=prtbs[:],
                                     initial=NEG_BIG, op0=OP.max, op1=OP.bypass)
        rtshb = smallp.tile([1, P], F32, tag="rtsh")
        nc.vector.memset(rtshb[:, 0:1], NEG_BIG)
        nc.vector.tensor_copy(out=rtshb[:, 1:], in_=rtsb[:, : P - 1])
        pc1b = pst.tile([P, 1], F32, tag="pt")
        nc.tensor.transpose(out=pc1b[:], in_=rtshb[:], identity=idf[:1, :1])
        carm = smallp.tile([P, 1], F32, tag="car")
        nc.vector.tensor_copy(out=carm[:], in_=pc1b[:])
        nc.vector.tensor_tensor_scan(out=dmx[:], data0=dmx[:], data1=dmx[:],
                                     initial=carm[:], op0=OP.max, op1=OP.bypass)
        nc.vector.tensor_tensor(out=cg[:], in0=cg[:], in1=dmx[:], op=OP.add)
        nc.scalar.activation(out=cg[:], in_=cg[:], func=ACTF.Exp)
        nc.vector.tensor_tensor(out=cg[:], in0=cg[:], in1=log1a[:],
                                op=OP.mult)

        # ---------- weighted per-ray sums ----------
        ta = cg[:]
        nc.vector.tensor_tensor(out=zc[:, :, 0:3], in0=zc[:, :, 0:3],
                                in1=_mk(ta, [*ta.ap, [0, 3]]), op=OP.mult)
        nc.vector.tensor_copy(out=zc[:, :, 3], in_=log1a[:])
        zrt = smallp.tile([P, 4], F32)
        for c in range(4):
            nc.vector.reduce_sum(out=zrt[:, c:c + 1], in_=zc[:, :, c], axis=AX.X)
        pzrt = pst.tile([4, P], F32, tag="pt")
        nc.tensor.transpose(out=pzrt[:], in_=zrt[:], identity=idf[:])
        pzrts = smallp.tile([4, P], F32)
        nc.vector.tensor_copy(out=pzrts[:], in_=pzrt[:])
        zcs = smallp.tile([4, P], F32)
        nc.vector.tensor_tensor_scan(out=zcs[:], data0=pzrts[:], data1=pzrts[:],
                                     initial=0.0, op0=OP.add, op1=OP.bypass)
        zsh = smallp.tile([4, P], F32)
        nc.vector.memset(zsh[:, 0:1], 0.0)
        nc.vector.tensor_copy(out=zsh[:, 1:], in_=zcs[:, : P - 1])
        pzc = pst.tile([P, 4], F32, tag="pt")
        nc.tensor.transpose(out=pzc[:], in_=zsh[:], identity=idf[:4, :4])
        zcar = smallp.tile([P, 4], F32)
        nc.vector.tensor_copy(out=zcar[:], in_=pzc[:])
        for c in range(4):
            nc.vector.tensor_tensor_scan(
                out=zc[:, :, c], data0=zc[:, :, c], data1=zc[:, :, c],
                initial=zcar[:, c:c + 1], op0=OP.add, op1=OP.bypass)
        d_z = nc.sync.dma_start(
            out=zdram.ap()[0:CAP, :].rearrange("(p f) c -> p (f c)", p=P),
            in_=zc[:])
        zpad = smallp.tile([8, 4], F32)
        nc.vector.memset(zpad[:], 0.0)
        d_zp = nc.sync.dma_start(out=zdram.ap()[CAP:CAP + 8, :], in_=zpad[:])
        ze = smallp.tile([P, RJ, 4], F32)
        zp = smallp.tile([P, RJ, 4], F32)
        for j in range(RJ):
            d_ge = nc.gpsimd.indirect_dma_start(
                out=ze[:, j, :], out_offset=None, in_=zdram.ap(),
                in_offset=bass.IndirectOffsetOnAxis(ap=epost[:, j:j + 1], axis=0))
            d_gp = nc.gpsimd.indirect_dma_start(
                out=zp[:, j, :], out_offset=None, in_=zdram.ap(),
                in_offset=bass.IndirectOffsetOnAxis(ap=ppost[:, j:j + 1], axis=0))
            for d_g in (d_ge, d_gp):
                add_dep_helper(d_g.ins, d_z.ins, reason="cumsum before gather")
                add_dep_helper(d_g.ins, d_zp.ins, reason="zpad before gather")
        ainv = smallp.tile([P, RJ], F32)
        nc.vector.tensor_tensor(out=ainv[:], in0=ze[:, :, 3], in1=zp[:, :, 3],
                                op=OP.subtract)
        nc.scalar.activation(out=ainv[:], in_=ainv[:], func=ACTF.Exp)
        outt = smallp.tile([P, RJ, 3], F32)
        nc.vector.tensor_tensor(out=outt[:], in0=zp[:, :, 0:3],
                                in1=ze[:, :, 0:3], op=OP.subtract)
        av = ainv[:]
        nc.vector.tensor_tensor(out=outt[:], in0=outt[:],
                                in1=_mk(av, [*av.ap, [0, 3]]), op=OP.add)
        nc.sync.dma_start(
            out=rgb_out.ap().rearrange("(j p) c -> p j c", p=P), in_=outt[:])

    nc.compile()
    return nc


# ================= host side =================

def prepare_in_maps(inputs, cfg: Cfg):
    xyz = np.asarray(inputs["xyz"], np.float32)
    ray_id = np.asarray(inputs["ray_id"]).astype(np.int64)
    viewdirs = np.asarray(inputs["viewdirs"], np.float32)
    mask = np.asarray(inputs["mask"])
    density = np.asarray(inputs["density"], np.float32)
    k0 = np.asarray(inputs["k0"], np.float32)
    n_rays = viewdirs.shape[0]
    ncores = n_rays // cfg.RPC
    assert ncores * cfg.RPC == n_rays

    g16 = np.zeros((GX, GY, GZ, NCH), dtype=ml_dtypes.bfloat16)
    g16[..., 0:NF] = k0
    g16[..., NF] = density
    g16[..., NF + 1] = mask.astype(np.float32)
    xe = np.minimum(np.arange(GX) + 1, GX - 1)
    ye = np.minimum(np.arange(GY) + 1, GY - 1)
    zi = np.minimum(np.arange(GZ) + 1, GZ - 1)
    grid = np.empty((GX, GY, GZ, 4, 2, NCH), dtype=ml_dtypes.bfloat16)
    grid[:, :, :, 0, 0] = g16
    grid[:, :, :, 0, 1] = g16[:, :, zi]
    grid[:, :, :, 1, 0] = g16[:, ye]
    grid[:, :, :, 1, 1] = g16[:, ye][:, :, zi]
    grid[:, :, :, 2, 0] = g16[xe]
    grid[:, :, :, 2, 1] = g16[xe][:, :, zi]
    grid[:, :, :, 3, 0] = g16[xe][:, ye]
    grid[:, :, :, 3, 1] = g16[xe][:, ye][:, :, zi]
    grid = grid.reshape(V, 8 * NCH)

    gb = np.searchsorted(ray_id, np.arange(n_rays + 1))
    w0 = np.asarray(inputs["W0"], np.float32)
    w1 = np.asarray(inputs["W1"], np.float32)
    w2 = np.asarray(inputs["W2"], np.float32)
    wr = np.asarray(inputs["Wr"], np.float32)
    br = np.asarray(inputs["br"], np.float32)
    wrh = np.zeros((WIDTH, 4), np.float32)
    wrh[:, 0:3] = wr[0:WIDTH, :]
    wrv = wr[WIDTH:, :]

    ident = np.eye(P, dtype=np.float32)
    f1 = (2.0 ** np.arange(POS_PE, dtype=np.float64) / (2 * np.pi)
          ).astype(np.float32)
    freqt = np.tile(np.repeat(f1[None, :], 3, axis=0).reshape(1, -1), (P, 1))
    f2 = (2.0 ** np.arange(VIEW_PE, dtype=np.float64) / (2 * np.pi)
          ).astype(np.float32)
    vfreqt = np.tile(np.repeat(f2[None, :], 3, axis=0).reshape(1, -1), (P, 1))
    szf = np.array([GX, GY, GZ], np.float32)

    common = dict(
        grid_in=grid,
        w0_in=w0.astype(ml_dtypes.bfloat16),
        w1_in=w1.astype(ml_dtypes.bfloat16),
        w2_in=w2.astype(ml_dtypes.bfloat16),
        wrh_in=wrh.astype(ml_dtypes.bfloat16),
        wrv_in=np.ascontiguousarray(wrv),
        brp_in=np.ascontiguousarray(br.reshape(3, 1)),
        idf_in=ident,
        idb_in=ident.astype(ml_dtypes.bfloat16),
        freq_in=freqt, vfreq_in=vfreqt,
        scl_in=np.tile(((szf - 1.0) / 2.0)[None, :], (P, 1)),
        tmax_in=np.tile((szf - 1.0)[None, :], (P, 1)),
        imax_in=np.tile((szf - 2.0)[None, :], (P, 1)),
        coef_in=np.tile(np.array([GY * GZ, GZ, 1.0], np.float32)[None, :],
                        (P, 1)),
    )

    def perm(a):  # ray r = j*128+p  ->  [P, RJ] int32
        return np.ascontiguousarray(
            np.asarray(a).reshape(cfg.RJ, P).T).astype(np.int32)

    in_maps = []
    for k in range(ncores):
        b0, b1 = int(gb[k * cfg.RPC]), int(gb[(k + 1) * cfg.RPC])
        nk = b1 - b0
        assert nk <= cfg.CAP, f"core {k}: {nk} samples > CAP {cfg.CAP}"
        xyzp = np.zeros((cfg.CAP, 3), np.float32)
        xyzp[:nk] = xyz[b0:b1]
        validm = np.zeros((cfg.CAP,), np.float32)
        validm[:nk] = -0.5
        rstart = (gb[k * cfg.RPC:(k + 1) * cfg.RPC] - b0).astype(np.int64)
        rend = (gb[k * cfg.RPC + 1:(k + 1) * cfg.RPC + 1] - b0).astype(np.int64)
        epos = rend - 1
        epos[epos < 0] = cfg.CAP
        ppos = rstart - 1
        ppos[ppos < 0] = cfg.CAP
        in_map = dict(common)
        in_map.update(
            xyz_in=xyzp, validm_in=validm,
            vd_in=np.ascontiguousarray(viewdirs[k * cfg.RPC:(k + 1) * cfg.RPC]),
            spos_in=perm(rstart), epos_in=perm(epos), ppos_in=perm(ppos),
        )
        in_maps.append(in_map)
    return in_maps


_PROG_CACHE = {}


def _get_prog(cfg: Cfg):
    key = (cfg.F, cfg.CH, cfg.RJ, cfg.GRP)
    if key not in _PROG_CACHE:
        _PROG_CACHE[key] = build_program(cfg)
    return _PROG_CACHE[key]


def run_on_hw(inputs, cfg=None, trace=False):
    from concourse import bass_utils
    if cfg is None:
        cfg = Cfg()
    nc = _get_prog(cfg)
    in_maps = prepare_in_maps(inputs, cfg)
    res = bass_utils.run_bass_kernel_spmd(
        nc, in_maps, core_ids=list(range(len(in_maps))), trace=trace)
    out = np.concatenate([r["rgb_out"] for r in res.results], axis=0)
    return out, res


def kernel(**inputs) -> np.ndarray:
    out, _ = run_on_hw(inputs)
    return out
